# revision 4
# baseline (speedup 1.0000x reference)
"""Trainium2 Bass kernel v2 for 1D morphological dilation (max-plus conv),
parabolic structuring element:

    out[i] = max_{k=-5..5} ( x[i+k] - k^2/(4*scale) ),  N = 2**24, f32.

Design (vs the int16 baseline):
  - Whole pipeline in float16. Host casts x -> fp16 (and a c1-pre-biased
    copy xb = fp16(x - 0.25/s)); output returns as fp16 and is widened on
    the host. Removes all three ScalarE f32->i16 conversion passes and
    halves input DMA vs f32.
  - Measured on HW: fp16 tensor_tensor runs in 2x_1P mode even at odd
    element offsets, so the +-1 pair max reads xb directly; no pre-shifted
    array needed.
  - DVE does exactly 9 tensor_tensor maxes (the minimal covering):
      n1  = max(xb[j-1], xb[j+1])              {+-1 @ c1}  (xb pre-biased)
      a1  = max(xq, n1)                        {0, +-1}
      v1  = n1 - (c3-c1)          (ACT)        {+-1 @ c3}
      n3  = max(v1[j-2], v1[j+2])              {+-3, +-1 @ c3}
      n2  = max(xq[j-2], xq[j+2])              {+-2 @ 0}
      a2  = max(a1, n3)
      v3  = n2 - c4               (ACT)        {+-2 @ c4}
      n2b = n2 - c2               (ACT, in place)
      a3  = max(a2, n2b)
      v2  = n3 - (c5-c3)          (ACT)        {+-3, +-1 @ c5}
      w   = max(v2, v3)
      wsh = max(w[j-2], w[j+2])                {+-4 @ c4, +-5 @ c5, dominated extras}
      out = max(a3, wsh)
    All four bias passes ride the otherwise-idle ScalarE (1 elem/cyc/lane);
    VectorE carries only the 9 maxes at 2 elem/cyc/lane.
  - fp16 error budget: cast 0.002 + two bias roundings 0.012 -> abs err
    <= ~0.014, rel err ~3e-3 (tolerance 2e-2).
"""

import os

import numpy as np

N = 16777216
N_CORES = 8
SHARD = N // N_CORES          # 2097152
P = 128
ROW = SHARD // P              # 16384
HALO = 6
PAD = -8.0                    # loses every max against the center tap

TILES = [256, 4992, 4992, 4992, 1152]
assert sum(TILES) == ROW

_CACHE = {}


def _build(scale, tiles=None, io_bufs=3, wk_bufs=3):
    import concourse.mybir as mybir
    from concourse import bacc, tile

    dt = mybir.dt
    Alu = mybir.AluOpType
    Act = mybir.ActivationFunctionType

    tiles = list(tiles) if tiles is not None else list(TILES)
    assert sum(tiles) == ROW
    fmax = max(tiles)
    AW = fmax + 2 * HALO + 4

    s = float(scale)
    c1 = 0.25 / s
    c2 = 1.0 / s
    c3 = 2.25 / s
    c4 = 4.0 / s
    c5 = 6.25 / s
    d13 = -(c3 - c1)
    d35 = -(c5 - c3)

    nc = bacc.Bacc()
    xq = nc.declare_dram_parameter("xq", [P, ROW + 2 * HALO], dt.float16, isOutput=False)
    xb = nc.declare_dram_parameter("xb", [P, ROW + 2 * HALO], dt.float16, isOutput=False)
    y = nc.declare_dram_parameter("y", [P, ROW], dt.float16, isOutput=True)

    with tile.TileContext(nc) as tc:
        with (
            tc.tile_pool(name="io", bufs=io_bufs) as io,
            tc.tile_pool(name="wk", bufs=wk_bufs) as wk,
        ):
            base = 0
            for f in tiles:
                W = f + 2 * HALO
                xqf = io.tile([P, AW], dt.float16)
                xbf = io.tile([P, AW], dt.float16)
                nc.sync.dma_start(xqf[:, 0:W], xq[:, base : base + W])
                nc.sync.dma_start(xbf[:, 0:W], xb[:, base : base + W])

                n1 = wk.tile([P, AW], dt.float16)
                n3 = wk.tile([P, AW], dt.float16)
                n2 = wk.tile([P, AW], dt.float16)
                v3 = wk.tile([P, AW], dt.float16)
                acc = xqf  # final accumulator lives in the xq tile

                # n1[j] = max(xb[j-1], xb[j+1]) over [2, f+10)
                nc.vector.tensor_tensor(
                    n1[:, 2 : f + 10], xbf[:, 1 : f + 9], xbf[:, 3 : f + 11], Alu.max
                )
                # n2[j] = max(xq[j-2], xq[j+2]) over [4, f+8), unbiased.
                # MUST read xqf before acc (aliased to xqf) is overwritten.
                nc.vector.tensor_tensor(
                    n2[:, 4 : f + 8], xqf[:, 2 : f + 6], xqf[:, 6 : f + 10], Alu.max
                )
                # acc = max(xq, n1) over [6, f+6)  (taps 0, +-1)
                nc.vector.tensor_tensor(
                    acc[:, 6 : f + 6], xqf[:, 6 : f + 6], n1[:, 6 : f + 6], Alu.max
                )
                # v1 = n1 - (c3-c1)  (ACT, in place; acc1 already consumed n1)
                nc.scalar.activation(
                    n1[:, 2 : f + 10], n1[:, 2 : f + 10], Act.Copy, bias=d13, scale=1.0
                )
                # n3[j] = max(v1[j-2], v1[j+2]) over [4, f+8)
                nc.vector.tensor_tensor(
                    n3[:, 4 : f + 8], n1[:, 2 : f + 6], n1[:, 6 : f + 10], Alu.max
                )
                # acc = max(acc, n3)
                nc.vector.tensor_tensor(
                    acc[:, 6 : f + 6], acc[:, 6 : f + 6], n3[:, 6 : f + 6], Alu.max
                )
                # v3 = n2 - c4 (ACT), then n2 -= c2 in place (ACT, ordered after)
                nc.scalar.activation(
                    v3[:, 4 : f + 8], n2[:, 4 : f + 8], Act.Copy, bias=-c4, scale=1.0
                )
                nc.scalar.activation(
                    n2[:, 4 : f + 8], n2[:, 4 : f + 8], Act.Copy, bias=-c2, scale=1.0
                )
                # acc = max(acc, n2 - c2)
                nc.vector.tensor_tensor(
                    acc[:, 6 : f + 6], acc[:, 6 : f + 6], n2[:, 6 : f + 6], Alu.max
                )
                # v2 = n3 - (c5-c3) (ACT, in place; acc2 already consumed n3)
                nc.scalar.activation(
                    n3[:, 4 : f + 8], n3[:, 4 : f + 8], Act.Copy, bias=d35, scale=1.0
                )
                # w = max(v2, v3) in place on n3; wsh = max(w[j-2], w[j+2]) into v3
                # (v3's last read is the w op, same engine, so no WAR hazard)
                nc.vector.tensor_tensor(
                    n3[:, 4 : f + 8], n3[:, 4 : f + 8], v3[:, 4 : f + 8], Alu.max
                )
                nc.vector.tensor_tensor(
                    v3[:, 6 : f + 6], n3[:, 4 : f + 4], n3[:, 8 : f + 8], Alu.max
                )
                nc.vector.tensor_tensor(
                    acc[:, 6 : f + 6], acc[:, 6 : f + 6], v3[:, 6 : f + 6], Alu.max
                )

                nc.sync.dma_start(y[:, base : base + f], acc[:, 6 : f + 6])
                base += f

    nc.compile()
    return nc


def _shard_inputs(x_full, scale):
    c1 = 0.25 / float(scale)
    padded_q = np.full(N + 2 * HALO, PAD, np.float16)
    padded_b = np.full(N + 2 * HALO, np.float16(PAD) - np.float16(c1), np.float16)
    padded_q[HALO : HALO + N] = x_full.astype(np.float16)
    padded_b[HALO : HALO + N] = (x_full - np.float32(c1)).astype(np.float16)
    in_maps = []
    for ci in range(N_CORES):
        maps = {}
        for name, padded in (("xq", padded_q), ("xb", padded_b)):
            sl = padded[ci * SHARD : ci * SHARD + SHARD + 2 * HALO]
            rows = np.lib.stride_tricks.as_strided(
                sl, shape=(P, ROW + 2 * HALO), strides=(2 * ROW, 2)
            )
            maps[name] = np.ascontiguousarray(rows)
        in_maps.append(maps)
    return in_maps


def kernel(input, scale):
    from concourse.bass_utils import run_bass_kernel_spmd

    x_full = np.ascontiguousarray(np.asarray(input, dtype=np.float32).reshape(N))
    key = float(np.asarray(scale))
    if key not in _CACHE:
        _CACHE[key] = _build(key)
    nc = _CACHE[key]

    trace = bool(os.environ.get("KERNEL_TRACE"))
    res = run_bass_kernel_spmd(
        nc,
        _shard_inputs(x_full, key),
        core_ids=list(range(N_CORES)),
        trace=trace,
    )
    kernel.last_exec_time_ns = res.exec_time_ns
    kernel.last_trace = res.instructions_and_trace
    kernel.last_profile_json = getattr(res, "profile_json", None)
    out = np.empty(N, dtype=np.float32)
    for ci in range(N_CORES):
        out[ci * SHARD : (ci + 1) * SHARD] = res.results[ci]["y"].reshape(-1)
    return out


kernel.last_exec_time_ns = None
kernel.last_trace = None
kernel.last_profile_json = None


# revision 5
# speedup vs baseline: 1.0336x; 1.0336x over previous
"""Trainium2 Bass kernel v2 for 1D morphological dilation (max-plus conv),
parabolic structuring element:

    out[i] = max_{k=-5..5} ( x[i+k] - k^2/(4*scale) ),  N = 2**24, f32.

Design (vs the int16 baseline):
  - Whole pipeline in float16. Host casts x -> fp16 (and a c1-pre-biased
    copy xb = fp16(x - 0.25/s)); output returns as fp16 and is widened on
    the host. Removes all three ScalarE f32->i16 conversion passes and
    halves input DMA vs f32.
  - Measured on HW: fp16 tensor_tensor runs in 2x_1P mode even at odd
    element offsets, so the +-1 pair max reads xb directly; no pre-shifted
    array needed.
  - DVE does exactly 9 tensor_tensor maxes (the minimal covering):
      n1  = max(xb[j-1], xb[j+1])              {+-1 @ c1}  (xb pre-biased)
      a1  = max(xq, n1)                        {0, +-1}
      v1  = n1 - (c3-c1)          (ACT)        {+-1 @ c3}
      n3  = max(v1[j-2], v1[j+2])              {+-3, +-1 @ c3}
      n2  = max(xq[j-2], xq[j+2])              {+-2 @ 0}
      a2  = max(a1, n3)
      v3  = n2 - c4               (ACT)        {+-2 @ c4}
      n2b = n2 - c2               (ACT, in place)
      a3  = max(a2, n2b)
      v2  = n3 - (c5-c3)          (ACT)        {+-3, +-1 @ c5}
      w   = max(v2, v3)
      wsh = max(w[j-2], w[j+2])                {+-4 @ c4, +-5 @ c5, dominated extras}
      out = max(a3, wsh)
    All four bias passes ride the otherwise-idle ScalarE (1 elem/cyc/lane);
    VectorE carries only the 9 maxes at 2 elem/cyc/lane.
  - fp16 error budget: cast 0.002 + two bias roundings 0.012 -> abs err
    <= ~0.014, rel err ~3e-3 (tolerance 2e-2).
"""

import os

import numpy as np

N = 16777216
N_CORES = 8
SHARD = N // N_CORES          # 2097152
P = 128
ROW = SHARD // P              # 16384
HALO = 6
PAD = -8.0                    # loses every max against the center tap

TILES = [1024, 2048, 4096, 4096, 4096, 1024]
assert sum(TILES) == ROW

_CACHE = {}


def _build(scale, tiles=None, io_bufs=3, wk_bufs=3):
    import concourse.mybir as mybir
    from concourse import bacc, tile

    dt = mybir.dt
    Alu = mybir.AluOpType
    Act = mybir.ActivationFunctionType

    tiles = list(tiles) if tiles is not None else list(TILES)
    assert sum(tiles) == ROW
    fmax = max(tiles)
    AW = fmax + 2 * HALO + 4

    s = float(scale)
    c1 = 0.25 / s
    c2 = 1.0 / s
    c3 = 2.25 / s
    c4 = 4.0 / s
    c5 = 6.25 / s
    d13 = -(c3 - c1)
    d35 = -(c5 - c3)

    nc = bacc.Bacc()
    xq = nc.declare_dram_parameter("xq", [P, ROW + 2 * HALO], dt.float16, isOutput=False)
    xb = nc.declare_dram_parameter("xb", [P, ROW + 2 * HALO], dt.float16, isOutput=False)
    y = nc.declare_dram_parameter("y", [P, ROW], dt.float16, isOutput=True)

    with tile.TileContext(nc) as tc:
        with (
            tc.tile_pool(name="io", bufs=io_bufs) as io,
            tc.tile_pool(name="wk", bufs=wk_bufs) as wk,
        ):
            base = 0
            for f in tiles:
                W = f + 2 * HALO
                xqf = io.tile([P, AW], dt.float16)
                xbf = io.tile([P, AW], dt.float16)
                nc.sync.dma_start(xqf[:, 0:W], xq[:, base : base + W])
                nc.sync.dma_start(xbf[:, 0:W], xb[:, base : base + W])

                n1 = wk.tile([P, AW], dt.float16)
                n3 = wk.tile([P, AW], dt.float16)
                n2 = wk.tile([P, AW], dt.float16)
                v2 = wk.tile([P, AW], dt.float16)
                v3 = wk.tile([P, AW], dt.float16)
                acc = xqf  # final accumulator lives in the xq tile

                # n1[j] = max(xb[j-1], xb[j+1]) over [2, f+10)
                nc.vector.tensor_tensor(
                    n1[:, 2 : f + 10], xbf[:, 1 : f + 9], xbf[:, 3 : f + 11], Alu.max
                )
                # n2[j] = max(xq[j-2], xq[j+2]) over [4, f+8), unbiased.
                # MUST read xqf before acc (aliased to xqf) is overwritten.
                nc.vector.tensor_tensor(
                    n2[:, 4 : f + 8], xqf[:, 2 : f + 6], xqf[:, 6 : f + 10], Alu.max
                )
                # acc = max(xq, n1) over [6, f+6)  (taps 0, +-1)
                nc.vector.tensor_tensor(
                    acc[:, 6 : f + 6], xqf[:, 6 : f + 6], n1[:, 6 : f + 6], Alu.max
                )
                # v1 = n1 - (c3-c1)  (ACT, in place; acc1 already consumed n1)
                nc.scalar.activation(
                    n1[:, 2 : f + 10], n1[:, 2 : f + 10], Act.Copy, bias=d13, scale=1.0
                )
                # n3[j] = max(v1[j-2], v1[j+2]) over [4, f+8)
                nc.vector.tensor_tensor(
                    n3[:, 4 : f + 8], n1[:, 2 : f + 6], n1[:, 6 : f + 10], Alu.max
                )
                # acc = max(acc, n3)
                nc.vector.tensor_tensor(
                    acc[:, 6 : f + 6], acc[:, 6 : f + 6], n3[:, 6 : f + 6], Alu.max
                )
                # v3 = n2 - c4 (ACT), then n2 -= c2 in place (ACT, ordered after)
                nc.scalar.activation(
                    v3[:, 4 : f + 8], n2[:, 4 : f + 8], Act.Copy, bias=-c4, scale=1.0
                )
                nc.scalar.activation(
                    n2[:, 4 : f + 8], n2[:, 4 : f + 8], Act.Copy, bias=-c2, scale=1.0
                )
                # acc = max(acc, n2 - c2)
                nc.vector.tensor_tensor(
                    acc[:, 6 : f + 6], acc[:, 6 : f + 6], n2[:, 6 : f + 6], Alu.max
                )
                # v2 = n3 - (c5-c3) (ACT)
                nc.scalar.activation(
                    v2[:, 4 : f + 8], n3[:, 4 : f + 8], Act.Copy, bias=d35, scale=1.0
                )
                # w = max(v2, v3) in place on v2; wsh = max(w[j-2], w[j+2]) into v3
                # (v3's last read is the w op, same engine, so no WAR hazard)
                nc.vector.tensor_tensor(
                    v2[:, 4 : f + 8], v2[:, 4 : f + 8], v3[:, 4 : f + 8], Alu.max
                )
                nc.vector.tensor_tensor(
                    v3[:, 6 : f + 6], v2[:, 4 : f + 4], v2[:, 8 : f + 8], Alu.max
                )
                nc.vector.tensor_tensor(
                    acc[:, 6 : f + 6], acc[:, 6 : f + 6], v3[:, 6 : f + 6], Alu.max
                )

                nc.sync.dma_start(y[:, base : base + f], acc[:, 6 : f + 6])
                base += f

    nc.compile()
    return nc


def _shard_inputs(x_full, scale):
    c1 = 0.25 / float(scale)
    padded_q = np.full(N + 2 * HALO, PAD, np.float16)
    padded_b = np.full(N + 2 * HALO, np.float16(PAD) - np.float16(c1), np.float16)
    padded_q[HALO : HALO + N] = x_full.astype(np.float16)
    padded_b[HALO : HALO + N] = (x_full - np.float32(c1)).astype(np.float16)
    in_maps = []
    for ci in range(N_CORES):
        maps = {}
        for name, padded in (("xq", padded_q), ("xb", padded_b)):
            sl = padded[ci * SHARD : ci * SHARD + SHARD + 2 * HALO]
            rows = np.lib.stride_tricks.as_strided(
                sl, shape=(P, ROW + 2 * HALO), strides=(2 * ROW, 2)
            )
            maps[name] = np.ascontiguousarray(rows)
        in_maps.append(maps)
    return in_maps


def kernel(input, scale):
    from concourse.bass_utils import run_bass_kernel_spmd

    x_full = np.ascontiguousarray(np.asarray(input, dtype=np.float32).reshape(N))
    key = float(np.asarray(scale))
    if key not in _CACHE:
        _CACHE[key] = _build(key)
    nc = _CACHE[key]

    trace = bool(os.environ.get("KERNEL_TRACE"))
    res = run_bass_kernel_spmd(
        nc,
        _shard_inputs(x_full, key),
        core_ids=list(range(N_CORES)),
        trace=trace,
    )
    kernel.last_exec_time_ns = res.exec_time_ns
    kernel.last_trace = res.instructions_and_trace
    kernel.last_profile_json = getattr(res, "profile_json", None)
    out = np.empty(N, dtype=np.float32)
    for ci in range(N_CORES):
        out[ci * SHARD : (ci + 1) * SHARD] = res.results[ci]["y"].reshape(-1)
    return out


kernel.last_exec_time_ns = None
kernel.last_trace = None
kernel.last_profile_json = None
